# revision 2
# baseline (speedup 1.0000x reference)
"""TRN2 Bass kernel for nn_Attention_87497073754296.

Computes, for Y [4096, 1024] f32 and W_param [1024, 1024] f32:
    G = Y @ W_param.T ; S = G @ G.T ; A = softmax(S, -1) ; Z = A @ Y
using the algebraic identity S = Y @ (W_param.T @ W_param) @ Y.T, so each
core only needs its own row-shard of the queries plus the (replicated)
full Y — no collectives.

Sharding: rows of Y (queries) are sharded 512/core across 8 cores.
M = W_param.T @ W_param (symmetric, d x d) is computed once on the host
and replicated.

Per core (q = 512 own queries):
    Ht  = (Yq @ M).T            via matmul(lhsT=M-block, rhs=Yq.T-block)
    S   = Ht.T @ Y.T            -> [512, 4096] row-block scores (bf16)
    P   = exp(S - rowmax)       ACT exp with accum_out row sums
    Pt  = P.T                   PE transposes (bf16, exact)
    Z   = (Pt.T @ (Yh+Ym+Yl)) * (1/rowsum)
where Yh/Ym/Yl is a 3-way bf16 split of Y: for the dominant (near
one-hot) softmax rows this reconstructs fp32-exact A @ Y, since
|y - (yh+ym+yl)| < 2^-27 |y| and PSUM accumulates in fp32.

All matmuls run in bf16 at 1 cycle/row on the PE.
"""
import numpy as np
import ml_dtypes

import concourse.bass as bass
import concourse.mybir as mybir
import concourse.tile as tile
from concourse import bacc
from concourse.bass_utils import run_bass_kernel_spmd
from concourse.masks import make_identity

F32 = mybir.dt.float32
BF16 = mybir.dt.bfloat16
AF = mybir.ActivationFunctionType
AX = mybir.AxisListType
OP = mybir.AluOpType

N, D = 4096, 1024
CORES = 8
QSH = N // CORES          # 512 queries per core
P = 128                   # partitions
DT = D // P               # 8 d-tiles
QT = QSH // P             # 4 q-tiles per core
JC = N // 512             # 8 j-chunks of 512 for scores
JT = N // P               # 32 j-tiles of 128 for A@Y
SPLITS = 3                # bf16 splits of Y for the final matmul

_CACHED = {}


def _build():
    nc = bacc.Bacc("TRN2", target_bir_lowering=False, debug=False,
                   num_devices=CORES)
    # DRAM parameters (per core). All bf16 except the f32 output.
    Mb = nc.declare_dram_parameter("Mb", [D, D], BF16, isOutput=False)
    Yqt = nc.declare_dram_parameter("Yqt", [D, QSH], BF16, isOutput=False)
    Yt = nc.declare_dram_parameter("Yt", [D, N], BF16, isOutput=False)
    Ysp = [
        nc.declare_dram_parameter(f"Ysp{s}", [N, D], BF16, isOutput=False)
        for s in range(SPLITS)
    ]
    Z = nc.declare_dram_parameter("Z", [QSH, D], F32, isOutput=True)

    with tile.TileContext(nc) as tc:
        with (
            tc.tile_pool(name="const", bufs=1) as const,
            tc.tile_pool(name="mpool", bufs=1) as mpool,
            tc.tile_pool(name="yqpool", bufs=1) as yqpool,
            tc.tile_pool(name="htpool", bufs=1) as htpool,
            tc.tile_pool(name="ytpool", bufs=2) as ytpool,
            tc.tile_pool(name="spool", bufs=1) as spool,
            tc.tile_pool(name="epool", bufs=2) as epool,
            tc.tile_pool(name="ptpool", bufs=1) as ptpool,
            tc.tile_pool(name="yzpool", bufs=2) as yzpool,
            tc.tile_pool(name="zopool", bufs=2) as zopool,
            tc.tile_pool(name="stat", bufs=1) as stat,
        ):
            ident = const.tile([P, P], BF16, name="ident")
            make_identity(nc, ident[:])

            # ---- load M and Yq^T ----
            m_sb = mpool.tile([P, DT * D], BF16, name="m_sb")
            for di in range(DT):
                nc.sync.dma_start(
                    m_sb[:, di * D:(di + 1) * D],
                    Mb[di * P:(di + 1) * P, :],
                )
            yq_sb = yqpool.tile([P, DT * QSH], BF16, name="yq_sb")
            for di in range(DT):
                nc.sync.dma_start(
                    yq_sb[:, di * QSH:(di + 1) * QSH],
                    Yqt[di * P:(di + 1) * P, :],
                )

            # ---- H phase: Ht[do, q] = sum_di M[di,do].T @ Yqt[di, q] ----
            ht_sb = htpool.tile([P, DT * QSH], BF16, name="ht_sb")
            with tc.tile_pool(name="psA", bufs=2, space="PSUM") as psA:
                for dt_ in range(DT):
                    hp = psA.tile([P, QSH], F32, name="hp", tag="h")
                    for di in range(DT):
                        nc.tensor.matmul(
                            hp[:],
                            m_sb[:, di * D + dt_ * P: di * D + (dt_ + 1) * P],
                            yq_sb[:, di * QSH:(di + 1) * QSH],
                            start=(di == 0), stop=(di == DT - 1),
                        )
                    nc.scalar.copy(ht_sb[:, dt_ * QSH:(dt_ + 1) * QSH], hp[:])

                # ---- S phase: S[t][q, j] (bf16 in SBUF) ----
                s_sb = [
                    spool.tile([P, N], BF16, name=f"s_sb{t}", tag=f"s{t}")
                    for t in range(QT)
                ]
                for jc in range(JC):
                    yt_sb = ytpool.tile([P, DT * 512], BF16, name="yt_sb")
                    for di in range(DT):
                        nc.sync.dma_start(
                            yt_sb[:, di * 512:(di + 1) * 512],
                            Yt[di * P:(di + 1) * P, jc * 512:(jc + 1) * 512],
                        )
                    for t in range(QT):
                        sp = psA.tile([P, 512], F32, name="sp", tag="s")
                        for di in range(DT):
                            nc.tensor.matmul(
                                sp[:],
                                ht_sb[:, di * QSH + t * P: di * QSH + (t + 1) * P],
                                yt_sb[:, di * 512:(di + 1) * 512],
                                start=(di == 0), stop=(di == DT - 1),
                            )
                        nc.scalar.copy(
                            s_sb[t][:, jc * 512:(jc + 1) * 512], sp[:]
                        )

                # ---- softmax + transpose phase ----
                st = stat.tile([P, 64], F32, name="st")
                negmax = st[:, 0:QT]
                recip = st[:, QT:2 * QT]
                rs8 = st[:, 8:8 + QT * JC]
                rowsum = st[:, 40:44]
                pt_sb = [
                    ptpool.tile([P, N], BF16, name=f"pt_sb{t}", tag=f"pt{t}")
                    for t in range(QT)
                ]
                for t in range(QT):
                    nc.vector.tensor_reduce(
                        negmax[:, t:t + 1], s_sb[t][:], axis=AX.X, op=OP.max,
                        negate=True,
                    )
                    for jc in range(JC):
                        e_sb = epool.tile([P, 512], BF16, name="e_sb")
                        nc.scalar.activation(
                            e_sb[:], s_sb[t][:, jc * 512:(jc + 1) * 512],
                            AF.Exp, bias=negmax[:, t:t + 1], scale=1.0,
                            accum_out=rs8[:, t * JC + jc: t * JC + jc + 1],
                        )
                        pp = psA.tile([P, 512], BF16, name="pp", tag="pt")
                        for k in range(4):
                            nc.tensor.transpose(
                                pp[:, k * P:(k + 1) * P],
                                e_sb[:, k * P:(k + 1) * P],
                                ident[:],
                            )
                        nc.vector.tensor_copy(
                            pt_sb[t][:, jc * 512:(jc + 1) * 512], pp[:]
                        )
                    nc.vector.tensor_reduce(
                        rowsum[:, t:t + 1], rs8[:, t * JC:(t + 1) * JC],
                        axis=AX.X, op=OP.add,
                    )
                    nc.vector.reciprocal(recip[:, t:t + 1], rowsum[:, t:t + 1])

            # ---- Z phase: Z[t] = (1/rowsum) * sum_j P^T[j,q].T @ Ysp[j,:] ----
            with tc.tile_pool(name="psZ", bufs=QT, space="PSUM") as psZ:
                zp = [
                    psZ.tile([P, D], F32, name=f"zp{t}", tag="z")
                    for t in range(QT)
                ]
                for jt in range(JT):
                    yz = yzpool.tile([P, SPLITS * D], BF16, name="yz")
                    for s in range(SPLITS):
                        nc.sync.dma_start(
                            yz[:, s * D:(s + 1) * D],
                            Ysp[s][jt * P:(jt + 1) * P, :],
                        )
                    for t in range(QT):
                        for dc in range(2):
                            for s in range(SPLITS):
                                nc.tensor.matmul(
                                    zp[t][:, dc * 512:(dc + 1) * 512],
                                    pt_sb[t][:, jt * P:(jt + 1) * P],
                                    yz[:, s * D + dc * 512: s * D + dc * 512 + 512],
                                    start=(jt == 0 and s == 0),
                                    stop=(jt == JT - 1 and s == SPLITS - 1),
                                )
                for t in range(QT):
                    zo = zopool.tile([P, D], F32, name="zo")
                    nc.scalar.activation(
                        zo[:], zp[t][:], AF.Copy, bias=0.0,
                        scale=recip[:, t:t + 1],
                    )
                    nc.sync.dma_start(Z[t * P:(t + 1) * P, :], zo[:])

    nc.finalize()
    return nc


def _bf16_split(x: np.ndarray, n: int) -> list[np.ndarray]:
    parts = []
    rem = np.ascontiguousarray(x, dtype=np.float32)
    for i in range(n):
        p = rem.astype(ml_dtypes.bfloat16)
        parts.append(p)
        if i < n - 1:
            rem = rem - p.astype(np.float32)
    return parts


def _prep_inputs(Y: np.ndarray, W_param: np.ndarray):
    Y = np.ascontiguousarray(Y, dtype=np.float32)
    W = np.ascontiguousarray(W_param, dtype=np.float32)
    M = (W.T @ W).astype(np.float32)
    Mb = M.astype(ml_dtypes.bfloat16)
    Ytb = np.ascontiguousarray(Y.T).astype(ml_dtypes.bfloat16)
    Ysp = _bf16_split(Y, SPLITS)
    in_maps = []
    for c in range(CORES):
        m = {
            "Mb": Mb,
            "Yqt": np.ascontiguousarray(Ytb[:, c * QSH:(c + 1) * QSH]),
            "Yt": Ytb,
        }
        for s in range(SPLITS):
            m[f"Ysp{s}"] = Ysp[s]
        in_maps.append(m)
    return in_maps


def _run(inputs: dict, trace: bool = False):
    Y = np.asarray(inputs["Y"])
    W = np.asarray(inputs["W_param"])
    assert Y.shape == (N, D) and W.shape == (D, D)
    if "nc" not in _CACHED:
        _CACHED["nc"] = _build()
    nc = _CACHED["nc"]
    in_maps = _prep_inputs(Y, W)
    res = run_bass_kernel_spmd(nc, in_maps, list(range(CORES)), trace=trace)
    out = np.concatenate(
        [res.results[c]["Z"] for c in range(CORES)], axis=0
    ).astype(np.float32)
    return out, res


def kernel(Y: np.ndarray, W_param: np.ndarray) -> np.ndarray:
    out, _ = _run({"Y": Y, "W_param": W_param})
    return out


# revision 4
# speedup vs baseline: 1.0665x; 1.0665x over previous
"""TRN2 Bass kernel for nn_Attention_87497073754296.

Computes, for Y [4096, 1024] f32 and W_param [1024, 1024] f32:
    G = Y @ W_param.T ; S = G @ G.T ; A = softmax(S, -1) ; Z = A @ Y
using the algebraic identity S = Y @ (W_param.T @ W_param) @ Y.T, so each
core only needs its own row-shard of the queries plus the (replicated)
full Y — no collectives.

Sharding: rows of Y (queries) are sharded 512/core across 8 cores.
M = W_param.T @ W_param (symmetric, d x d) is computed once on the host
and replicated.

Per core (q = 512 own queries):
    Ht  = (Yq @ M).T            via matmul(lhsT=M-block, rhs=Yq.T-block)
    S   = Ht.T @ Y.T            -> [512, 4096] row-block scores (bf16)
    P   = exp(S - rowmax)       ACT exp with accum_out row sums
    Pt  = P.T                   PE transposes (bf16, exact)
    Z   = (Pt.T @ (Yh+Ym+Yl)) * (1/rowsum)
where Yh/Ym/Yl is a 3-way bf16 split of Y: for the dominant (near
one-hot) softmax rows this reconstructs fp32-exact A @ Y, since
|y - (yh+ym+yl)| < 2^-27 |y| and PSUM accumulates in fp32.

All matmuls run in bf16 at 1 cycle/row on the PE.
"""
import numpy as np
import ml_dtypes

import concourse.bass as bass
import concourse.mybir as mybir
import concourse.tile as tile
from concourse import bacc
from concourse.bass_utils import run_bass_kernel_spmd
from concourse.masks import make_identity

F32 = mybir.dt.float32
BF16 = mybir.dt.bfloat16
AF = mybir.ActivationFunctionType
AX = mybir.AxisListType
OP = mybir.AluOpType

N, D = 4096, 1024
CORES = 8
QSH = N // CORES          # 512 queries per core
P = 128                   # partitions
DT = D // P               # 8 d-tiles
QT = QSH // P             # 4 q-tiles per core
JC = N // 512             # 8 j-chunks of 512 for scores
JT = N // P               # 32 j-tiles of 128 for A@Y
SPLITS = 3                # bf16 splits of Y for the final matmul

_CACHED = {}


def _build():
    nc = bacc.Bacc("TRN2", target_bir_lowering=False, debug=False,
                   num_devices=CORES)
    # DRAM parameters (per core). All bf16 except the f32 output.
    Mb = nc.declare_dram_parameter("Mb", [D, D], BF16, isOutput=False)
    Yqt = nc.declare_dram_parameter("Yqt", [D, QSH], BF16, isOutput=False)
    Yt = nc.declare_dram_parameter("Yt", [D, N], BF16, isOutput=False)
    Ysp = [
        nc.declare_dram_parameter(f"Ysp{s}", [N, D], BF16, isOutput=False)
        for s in range(SPLITS)
    ]
    Z = nc.declare_dram_parameter("Z", [QSH, D], F32, isOutput=True)

    with tile.TileContext(nc) as tc:
        with (
            tc.tile_pool(name="const", bufs=1) as const,
            tc.tile_pool(name="mpool", bufs=1) as mpool,
            tc.tile_pool(name="yqpool", bufs=1) as yqpool,
            tc.tile_pool(name="htpool", bufs=1) as htpool,
            tc.tile_pool(name="ytpool", bufs=2) as ytpool,
            tc.tile_pool(name="spool", bufs=1) as spool,
            tc.tile_pool(name="epool", bufs=2) as epool,
            tc.tile_pool(name="ptpool", bufs=1) as ptpool,
            tc.tile_pool(name="yzpool", bufs=2) as yzpool,
            tc.tile_pool(name="zopool", bufs=2) as zopool,
            tc.tile_pool(name="stat", bufs=1) as stat,
        ):
            ident = const.tile([P, P], BF16, name="ident")
            make_identity(nc, ident[:])

            # HAM warmup: keep the PE busy during the initial DMAs so the
            # clock gate is at 8/8 when real work arrives.
            with tc.tile_pool(name="warm", bufs=1, space="PSUM") as warm:
                wp = warm.tile([P, P], BF16, name="wp")
                for _ in range(96):
                    nc.tensor.transpose(wp[:], ident[:], ident[:])

            # ---- load M and Yq^T ----
            m_sb = mpool.tile([P, DT * D], BF16, name="m_sb")
            for di in range(DT):
                nc.sync.dma_start(
                    m_sb[:, di * D:(di + 1) * D],
                    Mb[di * P:(di + 1) * P, :],
                )
            yq_sb = yqpool.tile([P, DT * QSH], BF16, name="yq_sb")
            for di in range(DT):
                nc.sync.dma_start(
                    yq_sb[:, di * QSH:(di + 1) * QSH],
                    Yqt[di * P:(di + 1) * P, :],
                )

            # ---- H phase: Ht[do, q] = sum_di M[di,do].T @ Yqt[di, q] ----
            ht_sb = htpool.tile([P, DT * QSH], BF16, name="ht_sb")
            with tc.tile_pool(name="psA", bufs=2, space="PSUM") as psA:
                for dt_ in range(DT):
                    hp = psA.tile([P, QSH], F32, name="hp", tag="h")
                    for di in range(DT):
                        nc.tensor.matmul(
                            hp[:],
                            m_sb[:, di * D + dt_ * P: di * D + (dt_ + 1) * P],
                            yq_sb[:, di * QSH:(di + 1) * QSH],
                            start=(di == 0), stop=(di == DT - 1),
                        )
                    nc.scalar.copy(ht_sb[:, dt_ * QSH:(dt_ + 1) * QSH], hp[:])

                # stats tile: negmax/recip/rowsum [P, QT] + per-chunk maxes
                st = stat.tile([P, 64], F32, name="st")
                negmax = st[:, 0:QT]
                recip = st[:, QT:2 * QT]
                rowsum = st[:, 40:44]
                mx8 = st[:, 8:8 + QT * JC]

                # ---- S phase: S[t][q, j] (bf16 in SBUF) ----
                s_sb = [
                    spool.tile([P, N], BF16, name=f"s_sb{t}", tag=f"s{t}")
                    for t in range(QT)
                ]
                for jc in range(JC):
                    yt_sb = ytpool.tile([P, DT * 512], BF16, name="yt_sb")
                    for di in range(DT):
                        nc.sync.dma_start(
                            yt_sb[:, di * 512:(di + 1) * 512],
                            Yt[di * P:(di + 1) * P, jc * 512:(jc + 1) * 512],
                        )
                    for t in range(QT):
                        sp = psA.tile([P, 512], F32, name="sp", tag="s")
                        for di in range(DT):
                            nc.tensor.matmul(
                                sp[:],
                                ht_sb[:, di * QSH + t * P: di * QSH + (t + 1) * P],
                                yt_sb[:, di * 512:(di + 1) * 512],
                                start=(di == 0), stop=(di == DT - 1),
                            )
                        nc.scalar.copy(
                            s_sb[t][:, jc * 512:(jc + 1) * 512], sp[:]
                        )
                        nc.vector.tensor_reduce(
                            mx8[:, t * JC + jc: t * JC + jc + 1],
                            s_sb[t][:, jc * 512:(jc + 1) * 512],
                            axis=AX.X, op=OP.max,
                        )

                pt_sb = [
                    ptpool.tile([P, N], BF16, name=f"pt_sb{t}", tag=f"pt{t}")
                    for t in range(QT)
                ]
                for t in range(QT):
                    nc.vector.tensor_reduce(
                        negmax[:, t:t + 1], mx8[:, t * JC:(t + 1) * JC],
                        axis=AX.X, op=OP.max, negate=True,
                    )
                    e_sb = epool.tile([P, N], BF16, name="e_sb")
                    nc.scalar.activation(
                        e_sb[:], s_sb[t][:], AF.Exp,
                        bias=negmax[:, t:t + 1], scale=1.0,
                        accum_out=rowsum[:, t:t + 1],
                    )
                    nc.vector.reciprocal(recip[:, t:t + 1], rowsum[:, t:t + 1])
                    for jc in range(JC):
                        pp = psA.tile([P, 512], BF16, name="pp", tag="pt")
                        for k in range(4):
                            nc.tensor.transpose(
                                pp[:, k * P:(k + 1) * P],
                                e_sb[:, jc * 512 + k * P: jc * 512 + (k + 1) * P],
                                ident[:],
                            )
                        nc.vector.tensor_copy(
                            pt_sb[t][:, jc * 512:(jc + 1) * 512], pp[:]
                        )

            # ---- Z phase: Z[t] = (1/rowsum) * sum_j P^T[j,q].T @ Ysp[j,:] ----
            with tc.tile_pool(name="psZ", bufs=QT, space="PSUM") as psZ:
                zp = [
                    psZ.tile([P, D], F32, name=f"zp{t}", tag="z")
                    for t in range(QT)
                ]
                for jt in range(JT):
                    yz = yzpool.tile([P, SPLITS * D], BF16, name="yz")
                    for s in range(SPLITS):
                        nc.sync.dma_start(
                            yz[:, s * D:(s + 1) * D],
                            Ysp[s][jt * P:(jt + 1) * P, :],
                        )
                    for t in range(QT):
                        for dc in range(2):
                            for s in range(SPLITS):
                                nc.tensor.matmul(
                                    zp[t][:, dc * 512:(dc + 1) * 512],
                                    pt_sb[t][:, jt * P:(jt + 1) * P],
                                    yz[:, s * D + dc * 512: s * D + dc * 512 + 512],
                                    start=(jt == 0 and s == 0),
                                    stop=(jt == JT - 1 and s == SPLITS - 1),
                                )
                for t in range(QT):
                    zo = zopool.tile([P, D], F32, name="zo")
                    nc.scalar.activation(
                        zo[:], zp[t][:], AF.Copy, bias=0.0,
                        scale=recip[:, t:t + 1],
                    )
                    nc.sync.dma_start(Z[t * P:(t + 1) * P, :], zo[:])

    nc.finalize()
    return nc


def _bf16_split(x: np.ndarray, n: int) -> list[np.ndarray]:
    parts = []
    rem = np.ascontiguousarray(x, dtype=np.float32)
    for i in range(n):
        p = rem.astype(ml_dtypes.bfloat16)
        parts.append(p)
        if i < n - 1:
            rem = rem - p.astype(np.float32)
    return parts


def _prep_inputs(Y: np.ndarray, W_param: np.ndarray):
    Y = np.ascontiguousarray(Y, dtype=np.float32)
    W = np.ascontiguousarray(W_param, dtype=np.float32)
    M = (W.T @ W).astype(np.float32)
    Mb = M.astype(ml_dtypes.bfloat16)
    Ytb = np.ascontiguousarray(Y.T).astype(ml_dtypes.bfloat16)
    Ysp = _bf16_split(Y, SPLITS)
    in_maps = []
    for c in range(CORES):
        m = {
            "Mb": Mb,
            "Yqt": np.ascontiguousarray(Ytb[:, c * QSH:(c + 1) * QSH]),
            "Yt": Ytb,
        }
        for s in range(SPLITS):
            m[f"Ysp{s}"] = Ysp[s]
        in_maps.append(m)
    return in_maps


def _run(inputs: dict, trace: bool = False):
    Y = np.asarray(inputs["Y"])
    W = np.asarray(inputs["W_param"])
    assert Y.shape == (N, D) and W.shape == (D, D)
    if "nc" not in _CACHED:
        _CACHED["nc"] = _build()
    nc = _CACHED["nc"]
    in_maps = _prep_inputs(Y, W)
    res = run_bass_kernel_spmd(nc, in_maps, list(range(CORES)), trace=trace)
    out = np.concatenate(
        [res.results[c]["Z"] for c in range(CORES)], axis=0
    ).astype(np.float32)
    return out, res


def kernel(Y: np.ndarray, W_param: np.ndarray) -> np.ndarray:
    out, _ = _run({"Y": Y, "W_param": W_param})
    return out
